# revision 1
# baseline (speedup 1.0000x reference)
"""GNN mean-aggregation (PyG MessagePassing, aggr='mean') on 8 TRN2 NeuronCores.

Sharding strategy (host): edges are partitioned by destination across the 8
cores (core c owns dst in [12500c, 12500(c+1))), and within each core further
partitioned into 98 sub-shards by 128-node destination block. Edges keep
their original relative order inside each sub-shard; sub-shards are padded to
a uniform chunk count so one compiled program serves every round/direction.

Device program "aggregate" (per core, per round):
  - indirect-DMA gather of x[src] rows (128 rows / instruction)
  - one-hot build for dst-lo via DVE is_equal against an iota tile
  - PE matmul accumulates one-hot^T @ msg into a PSUM-resident accumulator
    [128 lo x (98 blocks x 16 dims)]
  - counts come from running the same program with x = ones
Device program "divide": out = sums * reciprocal(max(cnt, 1)).
Host only reassembles the 8 per-core output slices into the full h between
rounds (allgather equivalent).
"""
import sys
sys.path.insert(0, '/opt/trn_rl_repo')
import numpy as np

import concourse.bass as bass
import concourse.tile as tile
from concourse import bacc, mybir
from concourse.bass_utils import run_bass_kernel_spmd

N_NODES = 100000
DIM = 16
N_EDGES = 3200000
N_CORES = 8
NLOC = N_NODES // N_CORES        # 12500 nodes per core
NBLK = (NLOC + 127) // 128       # 98 blocks per core
PAD_LO = 128                     # one-hot sentinel (never matches iota 0..127)

_PROGRAMS = {}


def _shard(edge_index):
    """Partition edges by (core, dst-block); pad sub-shards to uniform U."""
    src = np.asarray(edge_index[0], dtype=np.int64)
    dst = np.asarray(edge_index[1], dtype=np.int64)
    core = dst // NLOC
    loc = dst - core * NLOC
    blk = loc // 128
    lo = loc % 128
    key = core * NBLK + blk
    order = np.argsort(key, kind='stable')
    ks, ss, ls = key[order], src[order], lo[order]
    counts = np.bincount(ks, minlength=N_CORES * NBLK).reshape(N_CORES, NBLK)
    U = int((counts.max() + 127) // 128)
    gsrc = np.zeros((N_CORES, NBLK * U * 128), np.int32)
    glo = np.full((N_CORES, NBLK * U * 128), PAD_LO, np.float32)
    starts = np.zeros(N_CORES * NBLK + 1, np.int64)
    np.cumsum(counts.ravel(), out=starts[1:])
    for c in range(N_CORES):
        for b in range(NBLK):
            k = c * NBLK + b
            n = counts[c, b]
            s0 = starts[k]
            base = (b * U) * 128
            gsrc[c, base:base + n] = ss[s0:s0 + n]
            glo[c, base:base + n] = ls[s0:s0 + n]
    return gsrc, glo, U


def _build_aggregate(U, counts_mode=False):
    NS = NBLK * U                      # chunk slots per core
    nc = bacc.Bacc("TRN2", target_bir_lowering=False, debug=False,
                   num_devices=N_CORES)
    h_in = nc.dram_tensor("h", [N_NODES, DIM], mybir.dt.float32,
                          kind="ExternalInput")
    gsrc = nc.dram_tensor("gsrc", [128, NS], mybir.dt.int32,
                          kind="ExternalInput")   # slot-major, wrapped to 128 partitions
    iotaf = nc.dram_tensor("iotaf", [128, 128], mybir.dt.float32,
                           kind="ExternalInput")
    glo = nc.dram_tensor("glo", [128, NS], mybir.dt.float32,
                         kind="ExternalInput")
    sums = nc.dram_tensor("sums", [128, NBLK * DIM], mybir.dt.float32,
                          kind="ExternalOutput")
    with tile.TileContext(nc) as tc:
        with (
            tc.tile_pool(name="const", bufs=1) as constp,
            tc.tile_pool(name="idx", bufs=1) as idxp,
            tc.tile_pool(name="msg", bufs=24) as msgp,
            tc.tile_pool(name="oh", bufs=24) as ohp,
            tc.tile_pool(name="accs", bufs=1) as accp,
            tc.tile_pool(name="psum", bufs=1, space="PSUM") as psump,
        ):
            iota = constp.tile([128, 128], mybir.dt.float32)
            nc.sync.dma_start(out=iota[:], in_=iotaf.ap()[:])
            idx_t = idxp.tile([128, NS], mybir.dt.int32)
            nc.sync.dma_start(out=idx_t[:], in_=gsrc.ap()[:])
            lo_t = idxp.tile([128, NS], mybir.dt.float32)
            nc.sync.dma_start(out=lo_t[:], in_=glo.ap()[:])
            ones_t = None
            if counts_mode:
                ones_t = constp.tile([128, DIM], mybir.dt.float32)
                nc.sync.dma_start(out=ones_t[:], in_=h_in.ap()[0:128, :])
            acc = psump.tile([128, NBLK * DIM], mybir.dt.float32, space="PSUM")
            for b in range(NBLK):
                for u in range(U):
                    s = b * U + u
                    if counts_mode:
                        msg = ones_t
                    else:
                        msg = msgp.tile([128, DIM], mybir.dt.float32, tag="msg")
                        nc.gpsimd.indirect_dma_start(
                            out=msg[:], out_offset=None, in_=h_in.ap()[:],
                            in_offset=bass.IndirectOffsetOnAxis(
                                ap=idx_t[:, s:s + 1], axis=0))
                    oh = ohp.tile([128, 128], mybir.dt.float32, tag="oh")
                    nc.vector.tensor_tensor(
                        out=oh[:], in0=lo_t[:, s:s + 1].to_broadcast([128, 128]),
                        in1=iota[:], op=mybir.AluOpType.is_equal)
                    nc.tensor.matmul(
                        out=acc[:, b * DIM:(b + 1) * DIM], lhsT=oh[:], rhs=msg[:],
                        start=(u == 0), stop=(u == U - 1))
            accs = accp.tile([128, NBLK * DIM], mybir.dt.float32)
            nc.vector.tensor_copy(out=accs[:], in_=acc[:])
            nc.sync.dma_start(out=sums.ap()[:], in_=accs[:])
    nc.compile()
    return nc


def _build_divide():
    nc = bacc.Bacc("TRN2", target_bir_lowering=False, debug=False,
                   num_devices=N_CORES)
    s_in = nc.dram_tensor("s", [128, NBLK * DIM], mybir.dt.float32, kind="ExternalInput")
    c_in = nc.dram_tensor("c", [128, NBLK * DIM], mybir.dt.float32, kind="ExternalInput")
    h_out = nc.dram_tensor("o", [128, NBLK * DIM], mybir.dt.float32, kind="ExternalOutput")
    with tile.TileContext(nc) as tc:
        with tc.tile_pool(name="p", bufs=2) as pool:
            st = pool.tile([128, NBLK * DIM], mybir.dt.float32, tag="s")
            nc.sync.dma_start(out=st[:], in_=s_in.ap()[:])
            ct = pool.tile([128, NBLK * DIM], mybir.dt.float32, tag="c")
            nc.sync.dma_start(out=ct[:], in_=c_in.ap()[:])
            cm = pool.tile([128, NBLK * DIM], mybir.dt.float32, tag="cm")
            nc.vector.tensor_scalar_max(out=cm[:], in0=ct[:], scalar1=1.0)
            cr = pool.tile([128, NBLK * DIM], mybir.dt.float32, tag="cr")
            nc.vector.reciprocal(out=cr[:], in_=cm[:])
            ot = pool.tile([128, NBLK * DIM], mybir.dt.float32, tag="o")
            nc.vector.tensor_mul(out=ot[:], in0=st[:], in1=cr[:])
            nc.sync.dma_start(out=h_out.ap()[:], in_=ot[:])
    nc.compile()
    return nc


def _wrap_slots(arr):
    # [NS*128] slot-major -> [128, NS] partition-wrapped (edge e of slot s at
    # partition e, column s)
    ns = arr.shape[-1] // 128
    return np.ascontiguousarray(arr.reshape(ns, 128).T)


def _run_aggregate(prog, h_full, gsrc_w, glo_w):
    core_ids = list(range(N_CORES))
    iota_np = np.tile(np.arange(128, dtype=np.float32), (128, 1))
    in_maps = [{"h": h_full, "gsrc": gsrc_w[c], "glo": glo_w[c], "iotaf": iota_np}
               for c in range(N_CORES)]
    res = run_bass_kernel_spmd(prog, in_maps, core_ids)
    return [res.results[c]["sums"] for c in range(N_CORES)]


def _run_divide(prog, sums_list, cnts_list):
    core_ids = list(range(N_CORES))
    in_maps = [{"s": sums_list[c], "c": cnts_list[c]} for c in range(N_CORES)]
    res = run_bass_kernel_spmd(prog, in_maps, core_ids)
    h = np.empty((N_NODES, DIM), np.float32)
    for c in range(N_CORES):
        o = res.results[c]["o"].reshape(128, NBLK, DIM).transpose(1, 0, 2)
        h[c * NLOC:(c + 1) * NLOC] = o.reshape(NBLK * 128, DIM)[:NLOC]
    return h


def kernel(topic_entity_one_hot, edge_index, reverse_edge_index):
    x = np.asarray(topic_entity_one_hot, dtype=np.float32)
    shards = [_shard(np.asarray(edge_index)),
              _shard(np.asarray(reverse_edge_index))]
    U = max(s[2] for s in shards)
    # re-shard with the common U so both directions fit one program
    def repad(ei):
        gsrc, glo, _ = _shard_fixed(np.asarray(ei), U)
        return gsrc, glo
    fwd = repad(edge_index)
    rev = repad(reverse_edge_index)

    if ("agg", U) not in _PROGRAMS:
        _PROGRAMS[("agg", U)] = _build_aggregate(U)
    if ("cnt", U) not in _PROGRAMS:
        _PROGRAMS[("cnt", U)] = _build_aggregate(U, counts_mode=True)
    if "div" not in _PROGRAMS:
        _PROGRAMS["div"] = _build_divide()
    agg, div = _PROGRAMS[("agg", U)], _PROGRAMS["div"]
    cntp = _PROGRAMS[("cnt", U)]

    results = []
    ones = np.ones((N_NODES, DIM), np.float32)
    for (gsrc, glo) in (fwd, rev):
        gsrc_w = [_wrap_slots(gsrc[c]) for c in range(N_CORES)]
        glo_w = [_wrap_slots(glo[c]) for c in range(N_CORES)]
        cnts = _run_aggregate(cntp, ones, gsrc_w, glo_w)
        h = x
        for _ in range(2):
            sums = _run_aggregate(agg, h, gsrc_w, glo_w)
            h = _run_divide(div, sums, cnts)
            results.append(h)
    out = np.stack([results[0], results[1], results[2], results[3]], axis=0)
    return out


def _shard_fixed(edge_index, U):
    src = np.asarray(edge_index[0], dtype=np.int64)
    dst = np.asarray(edge_index[1], dtype=np.int64)
    core = dst // NLOC
    loc = dst - core * NLOC
    blk = loc // 128
    lo = loc % 128
    key = core * NBLK + blk
    order = np.argsort(key, kind='stable')
    ks, ss, ls = key[order], src[order], lo[order]
    counts = np.bincount(ks, minlength=N_CORES * NBLK).reshape(N_CORES, NBLK)
    assert counts.max() <= U * 128
    gsrc = np.zeros((N_CORES, NBLK * U * 128), np.int32)
    glo = np.full((N_CORES, NBLK * U * 128), PAD_LO, np.float32)
    starts = np.zeros(N_CORES * NBLK + 1, np.int64)
    np.cumsum(counts.ravel(), out=starts[1:])
    for c in range(N_CORES):
        for b in range(NBLK):
            k = c * NBLK + b
            n = counts[c, b]
            s0 = starts[k]
            base = (b * U) * 128
            gsrc[c, base:base + n] = ss[s0:s0 + n]
            glo[c, base:base + n] = ls[s0:s0 + n]
    return gsrc, glo, U



# revision 7
# speedup vs baseline: 5.1189x; 5.1189x over previous
"""GNN mean-aggregation (PyG MessagePassing, aggr='mean') on 8 TRN2 NeuronCores.

Single fused device program per invocation:
  - 2 directions x 2 rounds of mean aggregation, counts, division, and the
    inter-round allgather all execute in ONE program launch.
  - Edges are dst-sharded across cores (core c owns dst in [12500c,12500(c+1)))
    and packed per 128-node dst block into slots of 128 edges, padded to a
    uniform U chunks per block (pad slots carry lo=128 so their one-hot row is
    zero and they contribute nothing, including to counts).
  - x is padded with a 17th all-ones column; the one-hot matmul then
    accumulates counts in the 17th accumulator column for free.
  - Hardware For_i loop over the 98 dst blocks; per iteration: one DMA stages
    the block's 34 offset columns from DRAM, one DVE is_equal builds all 34
    one-hots, then 34 (indirect-gather, matmul) pairs accumulate into PSUM.
  - Division on DVE (round 1 computes 1/max(cnt,1), round 2 reuses it); h1 is
    written to DRAM and allgathered across the 8 cores for round 2's gather.
Host only packs index metadata (static per graph) and reassembles outputs.
"""
import sys
sys.path.insert(0, '/opt/trn_rl_repo')
import numpy as np

import concourse.bass as bass
import concourse.tile as tile
from concourse import bacc, mybir
from concourse.bass import ds
from concourse.bass_utils import run_bass_kernel_spmd

N_NODES = 100000
DIM = 16
DIMP = DIM + 1                   # payload width: 16 dims + count column
DIMB = 32                        # PSUM stride per block (128B; never straddles a 2KB bank)
N_EDGES = 3200000
N_CORES = 8
NLOC = N_NODES // N_CORES        # 12500 dst nodes per core
NBLK = (NLOC + 127) // 128       # 98 blocks per core
RPAD = NBLK * 128                # 12544 padded rows per core
NFULL = N_CORES * RPAD           # 100352 rows in the replicated h layout
PAD_LO = 128.0                   # one-hot sentinel (never matches iota 0..127)

_PROGRAMS = {}


def _row_of_node(s):
    """Row of node s in the padded, partition-major replicated layout.

    Core c's slice is [128 partitions x 98 blocks], so node (c, l) with
    b=l//128, p=l%128 lives at row (c*128+p)*98 + b."""
    c, l = s // NLOC, s % NLOC
    return (c * 128 + l % 128) * NBLK + l // 128


def _shard_fixed(edge_index, U):
    """Per (core, dst-block) slots padded to U chunks.

    Returns gsrc [8, 128, NS] int32 (rows in the padded h layout) and
    glo [8, 128, NS] f32 (dst lo in 0..127, PAD_LO for padding)."""
    src = np.asarray(edge_index[0], dtype=np.int64)
    dst = np.asarray(edge_index[1], dtype=np.int64)
    core = dst // NLOC
    loc = dst - core * NLOC
    lo = loc % 128
    key = core * NBLK + loc // 128
    order = np.argsort(key, kind='stable')
    ss, ls = src[order], lo[order]
    counts = np.bincount(key, minlength=N_CORES * NBLK).reshape(N_CORES, NBLK)
    assert counts.max() <= U * 128, (counts.max(), U * 128)
    NS = NBLK * U
    rows = _row_of_node(ss).astype(np.int32)
    gsrc = np.zeros((N_CORES, NS * 128), np.int32)
    glo = np.full((N_CORES, NS * 128), PAD_LO, np.float32)
    starts = np.zeros(N_CORES * NBLK + 1, np.int64)
    np.cumsum(counts.ravel(), out=starts[1:])
    for c in range(N_CORES):
        for b in range(NBLK):
            k = c * NBLK + b
            n = counts[c, b]
            s0 = starts[k]
            base = (b * U) * 128
            gsrc[c, base:base + n] = rows[s0:s0 + n]
            glo[c, base:base + n] = ls[s0:s0 + n]
    # slot-major [NS*128] -> partition-wrapped [128, NS]
    gsrc = np.ascontiguousarray(gsrc.reshape(N_CORES, NS, 128).transpose(0, 2, 1))
    glo = np.ascontiguousarray(glo.reshape(N_CORES, NS, 128).transpose(0, 2, 1))
    return gsrc, glo


def _pack_x(x):
    """x [100000,16] -> padded partition-major layout [NFULL, 17]."""
    xp = np.zeros((NFULL, DIMP), np.float32)
    r = _row_of_node(np.arange(N_NODES))
    xp[r, :DIM] = x
    xp[r, DIM] = 1.0
    return xp


def _build_fused(U):
    NS = NBLK * U
    nc = bacc.Bacc("TRN2", target_bir_lowering=False, debug=False,
                   num_devices=N_CORES)
    xpad = nc.dram_tensor("xpad", [NFULL, DIMP], mybir.dt.float32,
                          kind="ExternalInput")
    gsrc = [nc.dram_tensor(f"gsrc{d}", [128, NS], mybir.dt.int32,
                           kind="ExternalInput") for d in range(2)]
    glo = [nc.dram_tensor(f"glo{d}", [128, NS], mybir.dt.float32,
                          kind="ExternalInput") for d in range(2)]
    iotaw = nc.dram_tensor("iotaw", [128, U * 128], mybir.dt.float32,
                           kind="ExternalInput")
    outs = [nc.dram_tensor(f"o{i}", [128, NBLK * DIM], mybir.dt.float32,
                           kind="ExternalOutput") for i in range(4)]
    hself = [nc.dram_tensor(f"hself{d}", [RPAD, DIMP], mybir.dt.float32)
             for d in range(2)]
    hfull = [nc.dram_tensor(f"hfull{d}", [NFULL, DIMP], mybir.dt.float32,
                            addr_space="Shared") for d in range(2)]

    with tile.TileContext(nc) as tc:
        with (
            tc.tile_pool(name="const", bufs=1) as constp,
            tc.tile_pool(name="stage", bufs=2) as stagep,
            tc.tile_pool(name="msg", bufs=2) as msgp,
            tc.tile_pool(name="oh", bufs=2) as ohp,
            tc.tile_pool(name="work", bufs=2) as workp,
            tc.tile_pool(name="psum", bufs=1, space="PSUM") as psump,
        ):
            iota = constp.tile([128, U, 128], mybir.dt.float32)
            nc.sync.dma_start(out=iota[:], in_=iotaw.ap()[:])
            lo_t = {}
            for d in range(2):
                llt = constp.tile([128, NS, 1], mybir.dt.float32, tag=f"lo{d}")
                nc.sync.dma_start(out=llt[:], in_=glo[d].ap()[:])
                lo_t[d] = llt

            for d in range(2):
                rinv = workp.tile([128, NBLK, 1], mybir.dt.float32,
                                  tag=f"rinv{d}")
                for r in range(2):
                    src = xpad if r == 0 else hfull[d]
                    acc = psump.tile([128, NBLK, DIMB], mybir.dt.float32,
                                     space="PSUM", tag="acc")
                    with tc.For_i(0, NBLK) as b:
                        jU = b * U
                        j17 = b * DIMB
                        gstage = stagep.tile([128, U], mybir.dt.int32,
                                             tag="gs")
                        nc.sync.dma_start(out=gstage[:],
                                          in_=gsrc[d].ap()[:, ds(jU, U)])
                        oh = ohp.tile([128, U, 128], mybir.dt.float32,
                                      tag="oh")
                        nc.vector.tensor_tensor(
                            out=oh[:],
                            in0=lo_t[d][:, ds(jU, U), :].to_broadcast(
                                [128, U, 128]),
                            in1=iota[:], op=mybir.AluOpType.is_equal)
                        for u in range(U):
                            msg = msgp.tile([128, DIMP], mybir.dt.float32,
                                            tag=f"msg{u % 4}")
                            nc.gpsimd.indirect_dma_start(
                                out=msg[:], out_offset=None,
                                in_=src.ap()[:, :],
                                in_offset=bass.IndirectOffsetOnAxis(
                                    ap=gstage[:, u:u + 1], axis=0))
                            nc.tensor.matmul(
                                out=acc[:, ds(b, 1), 0:DIMP],
                                lhsT=oh[:, u, :], rhs=msg[:],
                                start=(u == 0), stop=(u == U - 1))
                    h_sb = workp.tile([128, NBLK, DIMP], mybir.dt.float32,
                                      tag="h")
                    nc.vector.tensor_copy(out=h_sb[:], in_=acc[:, :, 0:DIMP])
                    if r == 0:
                        cnt_sb = workp.tile([128, NBLK, 1], mybir.dt.float32,
                                            tag="cnts")
                        nc.vector.tensor_scalar_max(
                            out=cnt_sb[:], in0=h_sb[:, :, DIM:DIMP],
                            scalar1=1.0)
                        nc.vector.reciprocal(out=rinv[:], in_=cnt_sb[:])
                    nc.vector.tensor_tensor(
                        out=h_sb[:], in0=h_sb[:],
                        in1=rinv[:].to_broadcast([128, NBLK, DIMP]),
                        op=mybir.AluOpType.mult)
                    nc.sync.dma_start(out=outs[2 * d + r].ap()[:],
                                      in_=h_sb[:, :, 0:DIM])
                    if r == 0:
                        nc.sync.dma_start(out=hself[d].ap()[:, :],
                                          in_=h_sb[:])
                        nc.gpsimd.collective_compute(
                            "AllGather", mybir.AluOpType.bypass,
                            replica_groups=[list(range(N_CORES))],
                            ins=[hself[d].ap().opt()],
                            outs=[hfull[d].ap().opt()])
    nc.compile()
    return nc


def _iota_np(U):
    i = np.tile(np.arange(128, dtype=np.float32), (128, U, 1))
    return np.ascontiguousarray(i.reshape(128, U * 128))


def _compute_U(edge_index, reverse_edge_index):
    U = 0
    for ei in (edge_index, reverse_edge_index):
        dst = np.asarray(ei[1], dtype=np.int64)
        key = (dst // NLOC) * NBLK + (dst % NLOC) // 128
        counts = np.bincount(key, minlength=N_CORES * NBLK)
        U = max(U, int((counts.max() + 127) // 128))
    return U


def make_inputs(topic_entity_one_hot, edge_index, reverse_edge_index, U):
    x = np.asarray(topic_entity_one_hot, dtype=np.float32)
    xp = _pack_x(x)
    gf, lf = _shard_fixed(np.asarray(edge_index), U)
    gr, lr = _shard_fixed(np.asarray(reverse_edge_index), U)
    iota = _iota_np(U)
    return [{"xpad": xp, "gsrc0": gf[c], "glo0": lf[c],
             "gsrc1": gr[c], "glo1": lr[c], "iotaw": iota}
            for c in range(N_CORES)]


def kernel(topic_entity_one_hot, edge_index, reverse_edge_index):
    U = _compute_U(edge_index, reverse_edge_index)
    if ("fused", U) not in _PROGRAMS:
        _PROGRAMS[("fused", U)] = _build_fused(U)
    prog = _PROGRAMS[("fused", U)]
    in_maps = make_inputs(topic_entity_one_hot, edge_index,
                          reverse_edge_index, U)
    res = run_bass_kernel_spmd(prog, in_maps, list(range(N_CORES)))
    out = np.empty((4, N_NODES, DIM), np.float32)
    for i in range(4):
        for c in range(N_CORES):
            o = res.results[c][f"o{i}"].reshape(128, NBLK, DIM)
            o = o.transpose(1, 0, 2).reshape(RPAD, DIM)[:NLOC]
            out[i, c * NLOC:(c + 1) * NLOC] = o
    return out
